# revision 1
# baseline (speedup 1.0000x reference)
"""BellmanFord GNN message-passing layer on 8 Trainium2 NeuronCores.

Reference computation (all f32):
    x   = h[src] + edge_rel_emb          # [E, D] gather
    hid = relu(x @ W1 + b1)              # [E, D]
    msg = hid @ W2 + b2                  # [E, D]
    agg = segment_sum(msg, dst, N)       # [N, D]
    out = h + agg

Strategy:
  - Shard edges across the 8 cores by *destination node range* (N/8 = 1250
    nodes per core) so each core owns its output slice outright -- no
    cross-core reduction needed.
  - Within a core, edges are sorted by dst and grouped into 128-node
    "windows"; each window's edges are padded to a multiple of 256 so all 8
    cores share one SPMD program (per-window tile counts = max over cores).
  - Per 256-edge macrotile: indirect-DMA gather (x = rel; x += h[src]),
    PE-transpose x -> xT, hidT = relu(W1^T xT + b1) and msgT = W2^T hidT + b2
    in the transposed domain (biases become per-partition ACT biases),
    PE-transpose msgT -> msg, then scatter-sum via a selection-matrix matmul
    (S[e, n] = [dst_local[e] == n]) accumulating into a per-window PSUM bank.
  - On window close: out = h + agg via DVE, DMA to the output slice.
  - Matmuls/transposes run in float32r (~1e-4 relative error, 4x faster than
    fp32 on the PE array).
"""

import sys

sys.path.insert(0, "/opt/trn_rl_repo")

import numpy as np

import concourse.bass as bass
import concourse.mybir as mybir
import concourse.tile as tile
from concourse import bacc
from concourse.bass_utils import run_bass_kernel_spmd
from concourse.masks import make_identity

P = 128
D = 256
N_CORES = 8
ET = 256  # edges per macrotile (2 x P)
WIN = P  # nodes per scatter window
F32 = mybir.dt.float32
F32R = mybir.dt.float32r
I32 = mybir.dt.int32
AF = mybir.ActivationFunctionType

_CACHE = {}
TRACE = False
TRACE_DIR = "/tmp/ktrace"


def _build_program(n_nodes, tiles_per_window, has_b1, has_b2):
    """Build the SPMD Bass program. Identical for all 8 cores.

    tiles_per_window: list of macrotile counts, one per 128-node window of the
    per-core node slice (node slice size = n_nodes // 8, padded to 128).
    """
    npc = n_nodes // N_CORES  # nodes per core
    n_win = len(tiles_per_window)
    n_tiles = int(sum(tiles_per_window))
    npc_pad = n_win * WIN

    nc = bacc.Bacc("TRN2", target_bir_lowering=False, debug=False,
                   num_devices=N_CORES)

    hs_d = nc.dram_tensor("h_slice", [npc_pad, D], F32,
                          kind="ExternalInput").ap()
    hsrc_d = nc.dram_tensor("hsrc", [n_tiles, P, 2, D], F32,
                            kind="ExternalInput").ap()
    rel_d = nc.dram_tensor("rel", [n_tiles, P, 2, D], F32,
                           kind="ExternalInput").ap()
    dst_d = nc.dram_tensor("dstT", [P, 2 * n_tiles], F32,
                           kind="ExternalInput").ap()
    w1_d = nc.dram_tensor("w1", [D, D], F32, kind="ExternalInput").ap()
    w2_d = nc.dram_tensor("w2", [D, D], F32, kind="ExternalInput").ap()
    b1_d = nc.dram_tensor("b1", [D], F32, kind="ExternalInput").ap()
    b2_d = nc.dram_tensor("b2", [D], F32, kind="ExternalInput").ap()
    out_d = nc.dram_tensor("out", [npc_pad, D], F32, kind="ExternalOutput").ap()

    with tile.TileContext(nc) as tc:
        with (
            tc.tile_pool(name="consts", bufs=1) as cb,
            tc.tile_pool(name="x", bufs=5) as x_pool,
            tc.tile_pool(name="xT", bufs=5) as xT_pool,
            tc.tile_pool(name="hidT", bufs=5) as hidT_pool,
            tc.tile_pool(name="msgT", bufs=5) as msgT_pool,
            tc.tile_pool(name="msg", bufs=5) as msg_pool,
            tc.tile_pool(name="S", bufs=5) as s_pool,
            tc.tile_pool(name="hw", bufs=3) as h_pool,
            tc.tile_pool(name="outw", bufs=3) as out_pool,
            tc.tile_pool(name="pstx", bufs=2, space="PSUM") as ps_tx,
            tc.tile_pool(name="pstm", bufs=2, space="PSUM") as ps_tm,
            tc.tile_pool(name="psm", bufs=2, space="PSUM") as ps_m,  # m1/m2 out
            tc.tile_pool(name="psA", bufs=2, space="PSUM") as ps_a,  # agg
        ):
            # ---- constants ----
            ident_f = cb.tile([P, P], F32)
            make_identity(nc, ident_f[:])
            ident_r = cb.tile([P, P], F32R)
            nc.vector.tensor_copy(ident_r[:], ident_f[:])

            iota_i = cb.tile([P, P], I32)
            nc.gpsimd.iota(iota_i[:], pattern=[[1, P]], base=0,
                           channel_multiplier=0)
            iota_f = cb.tile([P, P], F32)
            nc.vector.tensor_copy(iota_f[:], iota_i[:])

            # weights as lhsT layout [ki, ko, m] in f32r
            w1_f = cb.tile([P, 2, D], F32)
            nc.sync.dma_start(w1_f[:], w1_d.rearrange("(ko ki) m -> ki ko m",
                                                      ki=P))
            w1_r = cb.tile([P, 2, D], F32R)
            nc.vector.tensor_copy(w1_r[:], w1_f[:])
            w2_f = cb.tile([P, 2, D], F32)
            nc.sync.dma_start(w2_f[:], w2_d.rearrange("(ko ki) m -> ki ko m",
                                                      ki=P))
            w2_r = cb.tile([P, 2, D], F32R)
            nc.vector.tensor_copy(w2_r[:], w2_f[:])

            b1_sb = cb.tile([P, 2], F32)
            nc.sync.dma_start(b1_sb[:], b1_d.rearrange("(m p) -> p m", p=P))
            b2_sb = cb.tile([P, 2], F32)
            nc.sync.dma_start(b2_sb[:], b2_d.rearrange("(m p) -> p m", p=P))

            # all dstloc values in one DMA
            dst_sb = cb.tile([P, 2 * n_tiles], F32)
            nc.sync.dma_start(dst_sb[:], dst_d)

            # ---- software-pipelined emission ----
            # Stages (tile-lagged so every PE instr's inputs are one full
            # super-step old): dma -> add -> trx -> m1 -> m2 -> trm -> sc
            tile_win = []
            for w in range(n_win):
                tile_win += [w] * tiles_per_window[w]
            win_first = {}
            win_last = {}
            for ti, w in enumerate(tile_win):
                win_first.setdefault(w, ti)
                win_last[w] = ti
            T = n_tiles
            st = {}  # per-tile live tiles

            def s_dma(t):
                x_sb = x_pool.tile([P, 2, D], F32, name="x_sb")
                nc.sync.dma_start(x_sb[:], rel_d[t])
                g_sb = x_pool.tile([P, 2, D], F32, name="g_sb", tag="g")
                nc.sync.dma_start(g_sb[:], hsrc_d[t])
                st[t] = {"x": x_sb, "g": g_sb}

            def s_add(t):
                xr_sb = x_pool.tile([P, 2, D], F32R, name="xr_sb", tag="xr")
                nc.vector.tensor_add(xr_sb[:], st[t]["x"][:], st[t]["g"][:])
                st[t]["xr"] = xr_sb

            def s_trx(t):
                xr_sb = st[t]["xr"]
                xT_ps = ps_tx.tile([P, 2, ET], F32R, name="xT_ps")
                for j in range(2):
                    for k in range(2):
                        nc.tensor.transpose(
                            xT_ps[:, k, j * P:(j + 1) * P],
                            xr_sb[:, j, k * P:(k + 1) * P],
                            ident_r[:],
                        )
                xT_sb = xT_pool.tile([P, 2, ET], F32R)
                nc.scalar.copy(xT_sb[:], xT_ps[:])
                st[t]["xT"] = xT_sb

            def s_m1(t):
                hidT_ps = ps_m.tile([P, 2, ET], F32, name="hidT_ps",
                                    tag="mm")
                for m in range(2):
                    for k in range(2):
                        nc.tensor.matmul(
                            hidT_ps[:, m],
                            lhsT=w1_r[:, k, m * P:(m + 1) * P],
                            rhs=st[t]["xT"][:, k],
                            start=(k == 0),
                            stop=(k == 1),
                        )
                hidT_sb = hidT_pool.tile([P, 2, ET], F32R)
                if has_b1:
                    for m in range(2):
                        nc.scalar.activation(
                            hidT_sb[:, m], hidT_ps[:, m], AF.Relu,
                            bias=b1_sb[:, m:m + 1],
                        )
                else:
                    nc.scalar.activation(hidT_sb[:], hidT_ps[:], AF.Relu)
                st[t]["hidT"] = hidT_sb

            def s_m2(t):
                msgT_ps = ps_m.tile([P, 2, ET], F32, name="msgT_ps",
                                    tag="mm")
                for m in range(2):
                    for k in range(2):
                        nc.tensor.matmul(
                            msgT_ps[:, m],
                            lhsT=w2_r[:, k, m * P:(m + 1) * P],
                            rhs=st[t]["hidT"][:, k],
                            start=(k == 0),
                            stop=(k == 1),
                        )
                msgT_sb = msgT_pool.tile([P, 2, ET], F32R)
                if has_b2:
                    for m in range(2):
                        nc.vector.tensor_scalar(
                            out=msgT_sb[:, m], in0=msgT_ps[:, m],
                            scalar1=b2_sb[:, m:m + 1], scalar2=None,
                            op0=mybir.AluOpType.add,
                        )
                else:
                    nc.vector.tensor_copy(msgT_sb[:], msgT_ps[:])
                st[t]["msgT"] = msgT_sb

            def s_trm(t):
                msgT_sb = st[t]["msgT"]
                msg_ps = ps_tm.tile([P, 2, ET], F32R, name="msg_ps")
                for j in range(2):
                    for k in range(2):
                        nc.tensor.transpose(
                            msg_ps[:, j, k * P:(k + 1) * P],
                            msgT_sb[:, k, j * P:(j + 1) * P],
                            ident_r[:],
                        )
                msg_sb = msg_pool.tile([P, 2, ET], F32R)
                nc.scalar.copy(msg_sb[:], msg_ps[:])
                s_sb = s_pool.tile([P, 2, P], F32R, name="s_sb")
                for j in range(2):
                    nc.vector.tensor_scalar(
                        out=s_sb[:, j], in0=iota_f[:],
                        scalar1=dst_sb[:, 2 * t + j:2 * t + j + 1],
                        scalar2=None,
                        op0=mybir.AluOpType.is_equal,
                    )
                st[t]["msg"] = msg_sb
                st[t]["S"] = s_sb

            win_state = {}

            def s_sc(t):
                w = tile_win[t]
                if t == win_first[w]:
                    agg_ps = ps_a.tile([P, D], F32, name="agg_ps")
                    h_sb = h_pool.tile([P, D], F32, name="h_sb")
                    nc.sync.dma_start(h_sb[:],
                                      hs_d[w * WIN:(w + 1) * WIN])
                    win_state[w] = (agg_ps, h_sb)
                agg_ps, h_sb = win_state[w]
                first = (t == win_first[w])
                last = (t == win_last[w])
                for j in range(2):
                    nc.tensor.matmul(
                        agg_ps[:],
                        lhsT=st[t]["S"][:, j],
                        rhs=st[t]["msg"][:, j],
                        start=(first and j == 0),
                        stop=(last and j == 1),
                        skip_group_check=True,
                    )
                if last:
                    out_sb = out_pool.tile([P, D], F32, name="out_sb")
                    nc.vector.tensor_add(out_sb[:], agg_ps[:], h_sb[:])
                    nc.sync.dma_start(out_d[w * WIN:(w + 1) * WIN],
                                      out_sb[:])
                    del win_state[w]
                del st[t]

            # (stage, lag): every producer->consumer edge gets a 2-step
            # gap; PE consumes oldest data first (sc) and freshest last
            # (trx) to avoid in-order head-of-line blocking.
            stages = [(s_dma, 0), (s_add, 1), (s_sc, 10), (s_trm, 8),
                      (s_m2, 6), (s_m1, 4), (s_trx, 2)]
            L = 1 + max(lag for _, lag in stages)
            for i in range(T + L - 1):
                for fn, lag in stages:
                    t_ = i - lag
                    if 0 <= t_ < T:
                        fn(t_)

    nc.compile()
    return nc


def _prepare_shards(h, src, dst, rel, n_nodes):
    """Shard + sort + pad edges by destination range. Returns per-core input
    arrays and the shared tiles_per_window schedule."""
    npc = n_nodes // N_CORES
    n_win = (npc + WIN - 1) // WIN
    npc_pad = n_win * WIN

    cores = []
    counts = np.zeros((N_CORES, n_win), dtype=np.int64)
    for c in range(N_CORES):
        lo, hi = c * npc, (c + 1) * npc
        mask = (dst >= lo) & (dst < hi)
        idx = np.nonzero(mask)[0]
        d_c = dst[idx] - lo
        order = np.argsort(d_c, kind="stable")
        idx = idx[order]
        d_c = d_c[order]
        w_c = d_c // WIN
        counts[c] = np.bincount(w_c, minlength=n_win)
        cores.append((idx, d_c, w_c))

    tiles_per_window = [
        max(1, int(-(-counts[:, w].max() // ET))) for w in range(n_win)
    ]
    n_tiles = int(sum(tiles_per_window))
    starts = np.concatenate([[0], np.cumsum(tiles_per_window)])

    in_maps = []
    for c in range(N_CORES):
        idx, d_c, w_c = cores[c]
        src_pad = np.zeros(n_tiles * ET, dtype=np.int32)
        dloc_pad = np.full(n_tiles * ET, -1.0, dtype=np.float32)
        rel_pad = np.zeros((n_tiles * ET, D), dtype=np.float32)
        bounds = np.searchsorted(w_c, np.arange(n_win + 1))
        for w in range(n_win):
            a, b = bounds[w], bounds[w + 1]
            k = b - a
            off = int(starts[w]) * ET
            src_pad[off:off + k] = src[idx[a:b]]
            dloc_pad[off:off + k] = (d_c[a:b] - w * WIN).astype(np.float32)
            rel_pad[off:off + k] = rel[idx[a:b]]
        h_slice = np.zeros((npc_pad, D), dtype=np.float32)
        h_slice[:npc] = h[c * npc:(c + 1) * npc]
        rel_swz = np.ascontiguousarray(
            rel_pad.reshape(n_tiles, 2, P, D).transpose(0, 2, 1, 3))
        hsrc = np.ascontiguousarray(
            h[src_pad].reshape(n_tiles, 2, P, D).transpose(0, 2, 1, 3))
        in_maps.append({
            "h_slice": h_slice,
            "rel": rel_swz,
            "hsrc": hsrc,
            "dstT": np.ascontiguousarray(dloc_pad.reshape(2 * n_tiles, P).T),
        })
    return in_maps, tiles_per_window, npc, n_win


def kernel(h, edge_index, edge_rel_emb, W1, b1, W2, b2, num_nodes):
    h = np.ascontiguousarray(h, dtype=np.float32)
    rel = np.ascontiguousarray(edge_rel_emb, dtype=np.float32)
    W1 = np.ascontiguousarray(W1, dtype=np.float32)
    W2 = np.ascontiguousarray(W2, dtype=np.float32)
    b1 = np.ascontiguousarray(b1, dtype=np.float32)
    b2 = np.ascontiguousarray(b2, dtype=np.float32)
    n_nodes = int(num_nodes)
    src = np.asarray(edge_index[0]).astype(np.int64)
    dst = np.asarray(edge_index[1]).astype(np.int64)
    assert n_nodes % N_CORES == 0
    assert h.shape == (n_nodes, D)

    in_maps, tiles_per_window, npc, n_win = _prepare_shards(
        h, src, dst, rel, n_nodes)

    has_b1 = bool(np.any(b1))
    has_b2 = bool(np.any(b2))
    key = (n_nodes, tuple(tiles_per_window), has_b1, has_b2)
    if key not in _CACHE:
        _CACHE[key] = _build_program(n_nodes, tiles_per_window, has_b1, has_b2)
    nc = _CACHE[key]

    for m in in_maps:
        m["w1"] = W1
        m["w2"] = W2
        m["b1"] = b1
        m["b2"] = b2

    trace_kwargs = {}
    if TRACE:
        trace_kwargs = dict(trace=True, tmpdir=TRACE_DIR,
                            trace_cores=list(range(N_CORES)))
    res = run_bass_kernel_spmd(nc, in_maps, core_ids=list(range(N_CORES)),
                               **trace_kwargs)
    out = np.concatenate(
        [res.results[c]["out"][:npc] for c in range(N_CORES)], axis=0)

    # stash for test harnesses
    kernel.last_results = res
    return out.astype(np.float32)



# revision 2
# speedup vs baseline: 1.2683x; 1.2683x over previous
"""BellmanFord GNN message-passing layer on 8 Trainium2 NeuronCores.

Reference computation (all f32):
    x   = h[src] + edge_rel_emb          # [E, D] gather
    hid = relu(x @ W1 + b1)              # [E, D]
    msg = hid @ W2 + b2                  # [E, D]
    agg = segment_sum(msg, dst, N)       # [N, D]
    out = h + agg

Strategy (v3 = v2 + fp8 DoubleRow + engine rebalance):
  - Shard edges across the 8 cores by destination node range; each core owns
    its 1250-node output slice outright (no cross-core reduction).
  - Algebraic refactor: agg = (S^T relu(xW1)) W2 -- contract edges into nodes
    before the second matmul, so W2 runs per-node (32x less work) and the
    msg transpose disappears.
  - Host prep: sort edges by dst, 128-node windows, pad each window's edges
    to a multiple of 256 (shared SPMD schedule = max over cores), pre-add
    x = h[src] + rel, lay x out transposed per 128-edge tile in fp8e4m3, and
    precompute the one-hot scatter matrices S in fp8 as well.
  - Device per 128-edge tile: hid = relu(x @ W1) as ONE fp8 DoubleRow matmul
    (k=256 contracted in a single pass at 2x rate), relu evacuation rotated
    across Scalar/Pool/Vector engines (it is elem-paced, the single biggest
    non-PE cost), then per 256-edge pair Y_w += S^T hid as one fp8 DoubleRow
    matmul accumulating in PSUM over the window.
  - Per 128-node window close (bf16): Y -> bf16, PE-transpose,
    agg = Y^T-matmul W2, out = h + agg, DMA out.
  - fp8 end-to-end rel err ~7e-3 vs the 2e-2 gate (W2/finalize kept bf16).
"""

import sys

sys.path.insert(0, "/opt/trn_rl_repo")

import ml_dtypes
import numpy as np

import concourse.bass as bass
import concourse.mybir as mybir
import concourse.tile as tile
from concourse import bacc
from concourse.bass_utils import run_bass_kernel_spmd
from concourse.masks import make_identity

P = 128
D = 256
N_CORES = 8
WIN = P  # nodes per scatter window
G = 4  # tiles per relu batch (must be even: scatter runs on tile pairs)
LAG = 2  # batches between m1 and scatter consumption
BF16 = mybir.dt.bfloat16
FP8 = mybir.dt.float8e4
F32 = mybir.dt.float32
AF = mybir.ActivationFunctionType
DR = mybir.MatmulPerfMode.DoubleRow
NPBF16 = ml_dtypes.bfloat16
NPFP8 = ml_dtypes.float8_e4m3

_CACHE = {}
TRACE = False
TRACE_DIR = "/tmp/ktrace"


def _build_program(n_nodes, tiles_per_window, has_b1, has_b2):
    """Build the SPMD Bass program (identical for all 8 cores).

    tiles_per_window: 128-edge tile counts (all even), one per 128-node
    window of the per-core node slice.
    """
    npc = n_nodes // N_CORES
    n_win = len(tiles_per_window)
    T = int(sum(tiles_per_window))
    npc_pad = n_win * WIN
    kmax = max(tiles_per_window)
    starts = [0]
    for k in tiles_per_window:
        starts.append(starts[-1] + k)
    win_of = []
    for w, k in enumerate(tiles_per_window):
        win_of += [w] * k
    B = -(-T // G)  # relu batches

    nc = bacc.Bacc("TRN2", target_bir_lowering=False, debug=False,
                   num_devices=N_CORES)

    # xt: per-tile transposed x, [128 d-partition, T * (2 d-chunks * 128
    # edges)] fp8, window-contiguous per partition row.
    xt_d = nc.dram_tensor("xt", [P, T * 2 * P], FP8, kind="ExternalInput").ap()
    # st: one-hot scatter matrices per 256-edge pair, [128 edge-partition,
    # (T//2) * (2 edge-chunks * 128 node-cols)] fp8.
    st_d = nc.dram_tensor("st", [P, T * P], FP8, kind="ExternalInput").ap()
    hs_d = nc.dram_tensor("h_slice", [npc_pad, D], BF16,
                          kind="ExternalInput").ap()
    w1_d = nc.dram_tensor("w1x", [P, 2, D], FP8, kind="ExternalInput").ap()
    w2_d = nc.dram_tensor("w2x", [P, 2, D], BF16, kind="ExternalInput").ap()
    if has_b1:
        b1_d = nc.dram_tensor("b1bc", [P, D], F32, kind="ExternalInput").ap()
    if has_b2:
        b2_d = nc.dram_tensor("b2bc", [P, D], F32, kind="ExternalInput").ap()
        deg_d = nc.dram_tensor("degT", [P, n_win], F32,
                               kind="ExternalInput").ap()
    out_d = nc.dram_tensor("out", [npc_pad, D], F32, kind="ExternalOutput").ap()

    with tile.TileContext(nc) as tc:
        with (
            tc.tile_pool(name="consts", bufs=1) as cb,
            tc.tile_pool(name="wx", bufs=2) as wx_pool,
            tc.tile_pool(name="sw", bufs=2) as sw_pool,
            tc.tile_pool(name="hs", bufs=LAG + 2) as hs_pool,
            tc.tile_pool(name="ysb", bufs=2) as ysb_pool,
            tc.tile_pool(name="yts", bufs=2) as yts_pool,
            tc.tile_pool(name="hw", bufs=3) as hw_pool,
            tc.tile_pool(name="ow", bufs=2) as ow_pool,
            tc.tile_pool(name="hp", bufs=2, space="PSUM") as hp_pool,
            tc.tile_pool(name="yp", bufs=2, space="PSUM") as yp_pool,
            tc.tile_pool(name="ytp", bufs=1, space="PSUM") as ytp_pool,
            tc.tile_pool(name="agp", bufs=1, space="PSUM") as ag_pool,
        ):
            wstate = {}  # w -> (wx_tile, sw_tile, hw_tile)
            ystate = {}  # w -> Y psum tile
            bstate = {}  # b -> hid_sb
            pstate = {}  # b -> (hid psum, gn)

            def wdma(w, split_first=False):
                if w >= n_win:
                    return None
                k = tiles_per_window[w]
                wx_t = wx_pool.tile([P, kmax, 2, P], FP8, name="wx_t")
                src = xt_d[:, starts[w] * 2 * P:starts[w + 1] * 2 * P]
                src = src.rearrange("p (t c e) -> p t c e", t=k, c=2)
                half = None
                if split_first:
                    half = max(1, k // 2)
                    nc.sync.dma_start(wx_t[:, :half], src[:, :half])
                else:
                    nc.sync.dma_start(wx_t[:, :k], src)
                sw_t = sw_pool.tile([P, kmax // 2, 2, P], FP8, name="sw_t")
                ssrc = st_d[:, starts[w] * P:starts[w + 1] * P]
                ssrc = ssrc.rearrange("p (q c e) -> p q c e", q=k // 2, c=2)
                hw_t = hw_pool.tile([P, D], BF16, name="hw_t")
                wstate[w] = (wx_t, sw_t, hw_t)
                if split_first:
                    return (wx_t, src, half, k, sw_t, ssrc, hw_t, w)
                nc.sync.dma_start(sw_t[:, :k // 2], ssrc)
                nc.sync.dma_start(hw_t[:], hs_d[w * WIN:(w + 1) * WIN])
                return None

            def tiles_of(b):
                return range(G * b, min(G * b + G, T))

            def s_m1(b):
                gn = len(tiles_of(b))
                hp_t = hp_pool.tile([P, G, D], F32, name="hp_t")
                for g, t in enumerate(tiles_of(b)):
                    w = win_of[t]
                    if t == starts[w] and w >= 1:
                        wdma(w + 1)
                    wx_t = wstate[w][0]
                    trel = t - starts[w]
                    nc.tensor.matmul(
                        hp_t[:, g, :],
                        lhsT=wx_t[:, trel],
                        rhs=w1_sb[:],
                        start=True,
                        stop=True,
                        perf_mode=DR,
                    )
                pstate[b] = (hp_t, gn)

            def s_relu(b):
                hp_t, gn = pstate[b]
                if has_b1:
                    for g in range(gn):
                        nc.vector.tensor_add(hp_t[:, g, :], hp_t[:, g, :],
                                             b1_sb[:])
                hs_t = hs_pool.tile([P, G, D], FP8, name="hs_t")
                # GPSIMD/Pool cannot access PSUM on TRN2; alternate the
                # elem-paced evacuation between Scalar and Vector engines.
                if b % 2 == 0:
                    nc.scalar.activation(hs_t[:, :gn, :], hp_t[:, :gn, :],
                                         AF.Relu)
                else:
                    nc.vector.tensor_scalar(
                        out=hs_t[:, :gn, :], in0=hp_t[:, :gn, :],
                        scalar1=0.0, scalar2=None,
                        op0=mybir.AluOpType.max)
                bstate[b] = hs_t
                del pstate[b]

            def s_fin(w):
                yp_t = ystate.pop(w)
                hw_t = wstate.pop(w)[2]
                ysb_t = ysb_pool.tile([P, D], BF16, name="ysb_t")
                nc.scalar.copy(ysb_t[:], yp_t[:])
                ytp_t = ytp_pool.tile([P, 2, P], BF16, name="ytp_t")
                for c in range(2):
                    nc.tensor.transpose(ytp_t[:, c, :],
                                        ysb_t[:, c * P:(c + 1) * P],
                                        ident_b[:])
                yts_t = yts_pool.tile([P, 2, P], BF16, name="yts_t")
                nc.scalar.copy(yts_t[:], ytp_t[:])
                ag_t = ag_pool.tile([P, D], F32, name="ag_t")
                for c in range(2):
                    nc.tensor.matmul(
                        ag_t[:],
                        lhsT=yts_t[:, c, :],
                        rhs=w2_sb[:, c, :],
                        start=(c == 0),
                        stop=(c == 1),
                        skip_group_check=True,
                    )
                ow_t = ow_pool.tile([P, D], F32, name="ow_t")
                if has_b2:
                    nc.vector.tensor_scalar(
                        out=ow_t[:], in0=b2_sb[:],
                        scalar1=deg_sb[:, w:w + 1], scalar2=None,
                        op0=mybir.AluOpType.mult,
                    )
                    nc.vector.tensor_add(ow_t[:], ow_t[:], ag_t[:])
                    nc.vector.tensor_add(ow_t[:], ow_t[:], hw_t[:])
                else:
                    nc.vector.tensor_add(ow_t[:], ag_t[:], hw_t[:])
                nc.sync.dma_start(out_d[w * WIN:(w + 1) * WIN], ow_t[:])

            def s_sc(b):
                hs_t = bstate[b]
                for g, t in zip(range(0, G, 2), list(tiles_of(b))[::2]):
                    w = win_of[t]
                    if t == starts[w]:
                        yp_t = yp_pool.tile([P, D], F32, name="yp_t")
                        ystate[w] = yp_t
                    yp_t = ystate[w]
                    qrel = (t - starts[w]) // 2
                    nc.tensor.matmul(
                        yp_t[:],
                        lhsT=wstate[w][1][:, qrel],
                        rhs=hs_t[:, g:g + 2, :],
                        start=(t == starts[w]),
                        stop=(t == starts[w + 1] - 2),
                        perf_mode=DR,
                        skip_group_check=True,
                    )
                    if t == starts[w + 1] - 2:
                        s_fin(w)
                del bstate[b]

            # ---- startup: window-0 input first (split), weights threaded
            # between so the first m1 can issue as early as possible ----
            w0 = wdma(0, split_first=True)
            wx_t0, src0, half0, k0, sw_t0, ssrc0, hw_t0, _ = w0
            w1_sb = cb.tile([P, 2, D], FP8)
            nc.sync.dma_start(w1_sb[:], w1_d)
            nc.sync.dma_start(wx_t0[:, half0:k0], src0[:, half0:])
            nc.sync.dma_start(sw_t0[:, :k0 // 2], ssrc0)
            wdma(1)
            nc.sync.dma_start(hw_t0[:], hs_d[0:WIN])
            w2_sb = cb.tile([P, 2, D], BF16)
            nc.sync.dma_start(w2_sb[:], w2_d)
            if has_b1:
                b1_sb = cb.tile([P, D], F32)
                nc.sync.dma_start(b1_sb[:], b1_d)
            if has_b2:
                b2_sb = cb.tile([P, D], F32)
                nc.sync.dma_start(b2_sb[:], b2_d)
                deg_sb = cb.tile([P, n_win], F32)
                nc.sync.dma_start(deg_sb[:], deg_d)

            ident_f = cb.tile([P, P], F32)
            make_identity(nc, ident_f[:])
            ident_b = cb.tile([P, P], BF16)
            nc.vector.tensor_copy(ident_b[:], ident_f[:])

            # ---- software-pipelined emission ----
            for i in range(B + LAG):
                if 0 <= i - LAG < B:
                    s_sc(i - LAG)
                if i < B:
                    s_m1(i)
                if 0 <= i - 1 < B:
                    s_relu(i - 1)

    nc.compile()
    return nc


def _prepare_shards(h, src, dst, rel, n_nodes):
    """Shard + sort + pad edges by destination range; build per-core
    transposed pre-added fp8 edge inputs and fp8 one-hot scatter matrices.
    Returns per-core input dicts and the shared tiles_per_window schedule."""
    npc = n_nodes // N_CORES
    n_win = (npc + WIN - 1) // WIN
    npc_pad = n_win * WIN

    cores = []
    counts = np.zeros((N_CORES, n_win), dtype=np.int64)
    for c in range(N_CORES):
        lo, hi = c * npc, (c + 1) * npc
        mask = (dst >= lo) & (dst < hi)
        idx = np.nonzero(mask)[0]
        d_c = dst[idx] - lo
        order = np.argsort(d_c, kind="stable")
        idx = idx[order]
        d_c = d_c[order]
        w_c = d_c // WIN
        counts[c] = np.bincount(w_c, minlength=n_win)
        cores.append((idx, d_c, w_c))

    # tiles per window: even count of 128-edge tiles (scatter pairs tiles)
    tiles_per_window = [
        2 * max(1, int(-(-counts[:, w].max() // (2 * P))))
        for w in range(n_win)
    ]
    T = int(sum(tiles_per_window))
    starts = np.concatenate([[0], np.cumsum(tiles_per_window)])

    in_maps = []
    for c in range(N_CORES):
        idx, d_c, w_c = cores[c]
        x_pad = np.zeros((T * P, D), dtype=np.float32)
        dloc_pad = np.full(T * P, -1, dtype=np.int64)
        deg = np.zeros(npc_pad, dtype=np.float32)
        np.add.at(deg, d_c, 1.0)
        bounds = np.searchsorted(w_c, np.arange(n_win + 1))
        for w in range(n_win):
            a, b = bounds[w], bounds[w + 1]
            k = b - a
            off = int(starts[w]) * P
            e = idx[a:b]
            x_pad[off:off + k] = h[src[e]] + rel[e]
            dloc_pad[off:off + k] = d_c[a:b] - w * WIN
        h_slice = np.zeros((npc_pad, D), dtype=np.float32)
        h_slice[:npc] = h[c * npc:(c + 1) * npc]
        # xt[p, t, c, e] = x_pad[t*128+e, c*128+p]
        xt = np.ascontiguousarray(
            x_pad.reshape(T, P, 2, P).transpose(3, 0, 2, 1)
        ).reshape(P, T * 2 * P).astype(NPFP8)
        # S one-hot: s[p, q, i, n] = (dloc_pad[q*256 + i*128 + p] == n)
        sm = (dloc_pad.reshape(T // 2, 2, P)[:, :, :, None] ==
              np.arange(P)[None, None, None, :])
        st = np.ascontiguousarray(
            sm.transpose(2, 0, 1, 3)).reshape(P, T * P).astype(NPFP8)
        in_maps.append({
            "xt": xt,
            "st": st,
            "h_slice": h_slice.astype(NPBF16),
            "_deg": deg,
        })
    return in_maps, tiles_per_window, npc, n_win


def kernel(h, edge_index, edge_rel_emb, W1, b1, W2, b2, num_nodes):
    h = np.ascontiguousarray(h, dtype=np.float32)
    rel = np.ascontiguousarray(edge_rel_emb, dtype=np.float32)
    W1 = np.ascontiguousarray(W1, dtype=np.float32)
    W2 = np.ascontiguousarray(W2, dtype=np.float32)
    b1 = np.asarray(b1, dtype=np.float32)
    b2 = np.asarray(b2, dtype=np.float32)
    n_nodes = int(num_nodes)
    src = np.asarray(edge_index[0]).astype(np.int64)
    dst = np.asarray(edge_index[1]).astype(np.int64)
    assert n_nodes % N_CORES == 0
    assert h.shape == (n_nodes, D)

    in_maps, tiles_per_window, npc, n_win = _prepare_shards(
        h, src, dst, rel, n_nodes)

    has_b1 = bool(np.any(b1))
    has_b2 = bool(np.any(b2))
    key = (n_nodes, tuple(tiles_per_window), has_b1, has_b2)
    if key not in _CACHE:
        _CACHE[key] = _build_program(n_nodes, tiles_per_window, has_b1, has_b2)
    nc = _CACHE[key]

    w1x = np.ascontiguousarray(
        W1.reshape(2, P, D).transpose(1, 0, 2)).astype(NPFP8)
    w2x = np.ascontiguousarray(
        W2.reshape(2, P, D).transpose(1, 0, 2)).astype(NPBF16)
    for m in in_maps:
        deg = m.pop("_deg")
        m["w1x"] = w1x
        m["w2x"] = w2x
        if has_b1:
            m["b1bc"] = np.ascontiguousarray(
                np.broadcast_to(b1, (P, D))).astype(np.float32)
        if has_b2:
            m["b2bc"] = np.ascontiguousarray(
                np.broadcast_to(b2, (P, D))).astype(np.float32)
            m["degT"] = np.ascontiguousarray(
                deg.reshape(n_win, P).T).astype(np.float32)

    trace_kwargs = {}
    if TRACE:
        trace_kwargs = dict(trace=True, tmpdir=TRACE_DIR,
                            trace_cores=list(range(N_CORES)))
    res = run_bass_kernel_spmd(nc, in_maps, core_ids=list(range(N_CORES)),
                               **trace_kwargs)
    out = np.concatenate(
        [res.results[c]["out"][:npc] for c in range(N_CORES)], axis=0)

    kernel.last_results = res
    return out.astype(np.float32)
